# revision 1
# baseline (speedup 1.0000x reference)
"""Dynamic 3x3 per-pixel filter (DynamicFilterLayer2D) on 8 Trainium2 cores.

Reference: out[b,c,h,w] = sum_{i,j in 3x3} xpad[b,c,h+i,w+j] * f[b,c,(3i+j),h,w]

Sharding: H is split into 8 bands of 32 rows; each core processes all
(b, c) images for its band (data parallel, 1-row halo). Per-core layout:
partitions = 128 (b,c) images (2 groups of 128), free dim = flat pixels.

Compute: a custom DVE op `scan(ADD, Src0*Src1)` streams [pixel, j-tap]
pairs — x via an overlapping access pattern, filters host-interleaved to
[..., w, j] — producing a running sum of products; per-pixel 3-tap sums
are recovered by differencing the running sum at stride 3. Three such
scans (one per i row-tap) are combined with two adds, and one subtract
produces the output. Filter border columns (the taps that multiply
x-padding) are zeroed host-side, so x tiles need no column padding and
all access patterns have uniform strides.
"""

import numpy as np

B, C, H, W = 8, 32, 256, 256
K = 3
N_CORES = 8
BAND = H // N_CORES            # 32 rows per core
R = 4                          # output rows per compute sub-tile
RD = 8                         # rows per DMA super-tile
SUBS = RD // R                 # compute sub-tiles per super-tile (2)
N_SUPERS = BAND // RD          # 4
N_IMG = B * C                  # 256 images
P = 128
N_IMG_GROUPS = N_IMG // P      # 2
F = R * W                      # pixels per partition per sub-tile (1024)
FD = RD * W                    # pixels per partition per super-tile (2048)
X_SUPER = FD + 2 * W + 2       # x elements per super-tile (guards incl)
X_FLAT = (BAND + 2) * W + 2    # per-image padded x row storage

_CACHE = {}


def _register_mac_scan():
    from concourse import dve_ops
    from concourse.dve_ops import DveOp
    from concourse.dve_spec import Spec, Src0, Src1, scan, AluOp, lower
    from concourse.dve_uop import DveOpSpec

    name = "ANT_MAC_SCAN"
    for op in dve_ops.OPS:
        if op.name == name:
            return op

    def _ref(in0, in1, s0, s1, imm2):
        p = np.asarray(in0, np.float32) * np.asarray(in1, np.float32)
        flat = p.reshape(p.shape[0], -1)
        return np.cumsum(flat, axis=1, dtype=np.float32).reshape(p.shape)

    spec = Spec(body=scan(AluOp.ADD, Src0 * Src1), reference=_ref)
    op = DveOp(name, spec, False, {})
    dve_ops.OPS.append(op)
    dve_ops.CUSTOM_DVE_SPECS[name] = spec
    dve_ops._SUB_OPCODE_FOR_NAME[name] = (
        dve_ops._CUSTOM_DVE_ROW_BASE + len(dve_ops.OPS) - 1)
    for ver in ("v3", "v4"):
        dve_ops._COMPILE_CACHE[(name, ver)] = DveOpSpec(
            name=name,
            opcode=dve_ops.get_dve_sub_opcode(name),
            uops=lower(spec, ver=ver),
            rd1_en=True,
        )
    return op


def _strided_ap(tile_ap, dims, offset):
    """Copy of tile_ap with free dims replaced by [[step, count], ...]
    (element units) at element offset; partition dim preserved."""
    import bass_rust
    c = tile_ap.copy()
    part = list(c.ap)[0]
    c.ap = bass_rust.VecI64Pair([list(part)] + [list(d) for d in dims])
    c.offset = offset
    return c


def _build_module():
    import concourse.bacc as bacc
    import concourse.mybir as mybir
    from concourse.tile import TileContext

    mac_scan = _register_mac_scan()
    fp32 = mybir.dt.float32
    add = mybir.AluOpType.add
    sub = mybir.AluOpType.subtract

    nc = bacc.Bacc("TRN2", target_bir_lowering=False, debug=False)
    x_d = nc.dram_tensor("x_s", [N_IMG, X_FLAT], fp32,
                         kind="ExternalInput").ap()
    # host-interleaved filters: [img, i, band_row, w, j]
    f_d = nc.dram_tensor("f_s", [N_IMG, K, BAND, W, K], fp32,
                         kind="ExternalInput").ap()
    o_d = nc.dram_tensor("o_s", [N_IMG, BAND, W], fp32,
                         kind="ExternalOutput").ap()

    with TileContext(nc) as tc:
        with (
            tc.tile_pool(name="xp", bufs=2) as xpool,
            tc.tile_pool(name="fp", bufs=3) as fpool,
            tc.tile_pool(name="s0p", bufs=1) as s0pool,
            tc.tile_pool(name="s1p", bufs=1) as s1pool,
            tc.tile_pool(name="s2p", bufs=1) as s2pool,
            tc.tile_pool(name="vp", bufs=2) as vpool,
            tc.tile_pool(name="op", bufs=2) as opool,
        ):
            scpools = [s0pool, s1pool, s2pool]
            # per image-group list of (row_start, rows) super-tiles; the
            # schedule ends with two half supers so the compute backlog
            # after the final filter bytes arrive is halved
            supers = {
                g: [(t2 * RD, RD) for t2 in range(N_SUPERS)]
                for g in range(N_IMG_GROUPS)
            }
            supers[N_IMG_GROUPS - 1] = (
                [(t2 * RD, RD) for t2 in range(N_SUPERS - 1)]
                + [(BAND - RD, R), (BAND - R, 2), (BAND - 2, 1), (BAND - 1, 1)]
            )
            for g in range(N_IMG_GROUPS):
                for (r0, rd) in supers[g]:
                    p0 = g * P
                    fd = rd * W
                    n_subs = rd // R
                    xt = xpool.tile([P, X_SUPER], fp32, tag="x")
                    nc.gpsimd.dma_start(
                        out=xt[:, 0:fd + 2 * W + 2],
                        in_=x_d[p0:p0 + P, r0 * W: r0 * W + fd + 2 * W + 2],
                    )
                    fts = []
                    for i in range(K):
                        ft = fpool.tile([P, K * FD], fp32, tag="f", name="ft")
                        nc.sync.dma_start(
                            out=ft[:, 0:K * fd],
                            in_=f_d[p0:p0 + P, i, r0: r0 + rd, :, :],
                        )
                        fts.append(ft)
                    ot = opool.tile([P, FD], fp32, tag="o")
                    subs_list = []
                    sr = 0
                    while sr < rd:
                        rr = min(R, rd - sr)
                        subs_list.append((sr, rr))
                        sr += rr
                    for (sr, rr) in subs_list:
                        fs = rr * W       # pixels in this sub-tile
                        ps = sr * W       # local pixel start within super
                        vt = vpool.tile([P, F + 1], fp32, tag="v", name="vt")
                        nc.gpsimd.memset(vt[:, 0:1], 0.0)
                        scs = []
                        for i in range(K):
                            sct = scpools[i].tile([P, K * F], fp32,
                                                  tag=f"sc{i}", name="sct")
                            in0 = _strided_ap(xt[:, :], [[1, fs], [1, K]],
                                              ps + i * W)
                            in1 = _strided_ap(fts[i][:, :], [[K, fs], [1, K]],
                                              ps * K)
                            sc_out = _strided_ap(sct[:, :], [[K, fs], [1, K]],
                                                 0)
                            nc.vector._custom_dve(mac_scan, out=sc_out,
                                                  in0=in0, in1=in1)
                            scs.append(sct)
                        A = [_strided_ap(scs[i][:, :], [[K, fs]], K - 1)
                             for i in range(K)]
                        nc.vector.tensor_tensor(vt[:, 1:fs + 1], A[0], A[1],
                                                add)
                        nc.vector.tensor_tensor(vt[:, 1:fs + 1],
                                                vt[:, 1:fs + 1], A[2], add)
                        nc.vector.tensor_tensor(ot[:, ps:ps + fs],
                                                vt[:, 1:fs + 1], vt[:, 0:fs],
                                                sub)
                    # last (small) supers: HWDGE out skips the Q7 descriptor
                    # hop on the end-of-kernel critical chain
                    out_eng = nc.sync if rd < RD else nc.gpsimd
                    out_eng.dma_start(
                        out=o_d[p0:p0 + P, r0:r0 + rd, :],
                        in_=ot[:, 0:fd],
                    )
    nc.compile()
    return nc


def _get_module():
    if "nc" not in _CACHE:
        _CACHE["nc"] = _build_module()
    return _CACHE["nc"]


def _shard_inputs(x, dynamic_filters):
    """Per-core input maps. x: [B,C,H,W] f32, filters: [B,C*9,H,W] f32."""
    xp = np.pad(x, ((0, 0), (0, 0), (1, 1), (0, 0)))   # pad rows only
    # filters -> [B, C, i, j, H, W] -> zero border cols -> [img, i, H, W, j]
    f6 = dynamic_filters.reshape(B, C, K, K, H, W).copy()
    f6[:, :, :, 0, :, 0] = 0.0      # j=0 taps multiply x col -1
    f6[:, :, :, 2, :, W - 1] = 0.0  # j=2 taps multiply x col W
    f_int = np.ascontiguousarray(
        f6.transpose(0, 1, 2, 4, 5, 3)).reshape(N_IMG, K, H, W, K)

    in_maps = []
    for n in range(N_CORES):
        r = n * BAND
        xs = xp[:, :, r:r + BAND + 2, :].reshape(N_IMG, (BAND + 2) * W)
        xs_flat = np.zeros((N_IMG, X_FLAT), np.float32)
        xs_flat[:, 1:-1] = xs
        fs = np.ascontiguousarray(f_int[:, :, r:r + BAND])
        in_maps.append({"x_s": xs_flat, "f_s": fs})
    return in_maps


def kernel(x, dynamic_filters, _trace=False):
    from concourse import bass_utils

    x = np.asarray(x, dtype=np.float32)
    dynamic_filters = np.asarray(dynamic_filters, dtype=np.float32)
    nc = _get_module()
    in_maps = _shard_inputs(x, dynamic_filters)
    res = bass_utils.run_bass_kernel_spmd(
        nc, in_maps, list(range(N_CORES)), trace=_trace)
    out = np.concatenate(
        [res.results[n]["o_s"].reshape(B, C, BAND, W) for n in range(N_CORES)],
        axis=2)
    _CACHE["last_exec_time_ns"] = res.exec_time_ns
    return out



# revision 2
# speedup vs baseline: 1.1558x; 1.1558x over previous
"""Dynamic 3x3 per-pixel filter (DynamicFilterLayer2D) on 8 Trainium2 cores.

Reference: out[b,c,h,w] = sum_{i,j in 3x3} xpad[b,c,h+i,w+j] * f[b,c,(3i+j),h,w]

Sharding: H is split into 8 bands of 32 rows; each core processes all
(b, c) images for its band (data parallel, 1-row halo). Per-core layout:
partitions = 128 (b,c) images (2 groups of 128), free dim = flat pixels.

All HBM traffic is fp16 (the 2e-2 rel-err gate leaves ~20x margin), which
halves DMA time versus fp32; DMA is the roofline at ~371 GB/s/core.

Compute (measured rates on HW): fp16 tensor_tensor with fully packed APs
runs in DVE 2x mode (~0.53 ns/elem vs 1.05 at 1x). So the kernel works in
planar tap layout: filters are staged as 9 contiguous per-tap planes, and
each tap's product is one contiguous multiply (the 3x3 window shift is
just an offset into the row-flat x tile). The 9 products are summed with
6 adds on DVE and 2 on GPSIMD (load balancing), all contiguous fp16.
Filter border columns (taps that would read x column padding) are zeroed
host-side, so column wrap reads multiply garbage by 0 and no x column
padding is needed.
"""

import numpy as np

B, C, H, W = 8, 32, 256, 256
K = 3
N_CORES = 8
BAND = H // N_CORES            # 32 rows per core
RD = 8                         # rows per super-tile
NSUP = BAND // RD              # 4
FS = RD * W                    # pixels per partition per super-tile (2048)
N_IMG = B * C                  # 256 images
P = 128
GROUPS = N_IMG // P            # 2
XLEN = (BAND + 2) * W + 2      # per-image padded x row storage (8706)

_CACHE = {}


def _build_module():
    import concourse.bacc as bacc
    import concourse.mybir as mybir
    from concourse.tile import TileContext

    fp16 = mybir.dt.float16
    add = mybir.AluOpType.add
    mult = mybir.AluOpType.mult

    nc = bacc.Bacc("TRN2", target_bir_lowering=False, debug=False)
    x_d = nc.dram_tensor("x_s", [N_IMG, XLEN], fp16,
                         kind="ExternalInput").ap()
    # planar taps: [img, tap, band_row, w]
    f_d = nc.dram_tensor("f_s", [N_IMG, K * K, BAND, W], fp16,
                         kind="ExternalInput").ap()
    o_d = nc.dram_tensor("o_s", [N_IMG, BAND, W], fp16,
                         kind="ExternalOutput").ap()

    with TileContext(nc) as tc:
        with (
            tc.tile_pool(name="xp", bufs=2) as xpool,
            tc.tile_pool(name="fp", bufs=2) as fpool,
            tc.tile_pool(name="pp", bufs=1) as ppool,
            tc.tile_pool(name="bp", bufs=2) as bpool,
            tc.tile_pool(name="op", bufs=2) as opool,
        ):
            for g in range(GROUPS):
                p0 = g * P
                xt = xpool.tile([P, XLEN], fp16, tag="x")
                nc.sync.dma_start(out=xt[:, :], in_=x_d[p0:p0 + P, :])
                for s in range(NSUP):
                    r0 = s * RD
                    ft = fpool.tile([P, K * K * FS], fp16, tag="f")
                    nc.sync.dma_start(
                        out=ft[:, :],
                        in_=f_d[p0:p0 + P, :, r0:r0 + RD, :],
                    )
                    pt = ppool.tile([P, K * K * FS], fp16, tag="p")
                    for t in range(K * K):
                        i, j = divmod(t, K)
                        off = r0 * W + i * W + j
                        nc.vector.tensor_tensor(
                            pt[:, t * FS:(t + 1) * FS],
                            xt[:, off:off + FS],
                            ft[:, t * FS:(t + 1) * FS],
                            mult,
                        )
                    # DVE sums taps 0..6 in place; GPSIMD does 7+8 and the
                    # final combine (2x fp16 DVE ~16us/super, gpsimd ~13us)
                    at = pt[:, 0:FS]
                    for t in range(1, 7):
                        nc.vector.tensor_tensor(
                            at, at, pt[:, t * FS:(t + 1) * FS], add)
                    bt = bpool.tile([P, FS], fp16, tag="b")
                    nc.gpsimd.tensor_tensor(
                        bt[:, :], pt[:, 7 * FS:8 * FS], pt[:, 8 * FS:9 * FS],
                        add)
                    ot = opool.tile([P, FS], fp16, tag="o")
                    nc.gpsimd.tensor_tensor(ot[:, :], at, bt[:, :], add)
                    nc.scalar.dma_start(
                        out=o_d[p0:p0 + P, r0:r0 + RD, :],
                        in_=ot[:, :],
                    )
    nc.compile()
    return nc


def _get_module():
    if "nc" not in _CACHE:
        _CACHE["nc"] = _build_module()
    return _CACHE["nc"]


def _shard_inputs(x, dynamic_filters):
    """Per-core input maps. x: [B,C,H,W] f32, filters: [B,C*9,H,W] f32."""
    xp = np.pad(x, ((0, 0), (0, 0), (1, 1), (0, 0))).astype(np.float16)
    # planar taps [img, t, H, W]; zero border cols (j=0 @ w=0, j=2 @ w=W-1)
    fp = np.ascontiguousarray(
        dynamic_filters.reshape(N_IMG, K * K, H, W)).astype(np.float16)
    fp[:, 0::3, :, 0] = 0.0
    fp[:, 2::3, :, W - 1] = 0.0

    in_maps = []
    for n in range(N_CORES):
        r = n * BAND
        xs = xp[:, :, r:r + BAND + 2, :].reshape(N_IMG, (BAND + 2) * W)
        xs_flat = np.zeros((N_IMG, XLEN), np.float16)
        xs_flat[:, 1:-1] = xs
        fs = np.ascontiguousarray(fp[:, :, r:r + BAND])
        in_maps.append({"x_s": xs_flat, "f_s": fs})
    return in_maps


def kernel(x, dynamic_filters, _trace=False):
    from concourse import bass_utils

    x = np.asarray(x, dtype=np.float32)
    dynamic_filters = np.asarray(dynamic_filters, dtype=np.float32)
    nc = _get_module()
    in_maps = _shard_inputs(x, dynamic_filters)
    res = bass_utils.run_bass_kernel_spmd(
        nc, in_maps, list(range(N_CORES)), trace=_trace)
    out = np.concatenate(
        [res.results[n]["o_s"].reshape(B, C, BAND, W).astype(np.float32)
         for n in range(N_CORES)],
        axis=2)
    _CACHE["last_exec_time_ns"] = res.exec_time_ns
    return out


# revision 3
# speedup vs baseline: 1.5075x; 1.3044x over previous
"""Dynamic 3x3 per-pixel filter (DynamicFilterLayer2D) on 8 Trainium2 cores.

Reference: out[b,c,h,w] = sum_{i,j in 3x3} xpad[b,c,h+i,w+j] * f[b,c,(3i+j),h,w]

Sharding: H is split into 8 bands of 32 rows; each core processes all
(b, c) images for its band (data parallel, 1-row halo). Per-core layout:
partitions = 128 (b,c) images (2 groups of 128), free dim = flat pixels.

All HBM traffic is fp16 (the 2e-2 rel-err gate leaves ~20x margin), which
halves DMA bytes versus fp32; per-core DMA roofline is ~371 GB/s.

Compute is pure DVE in 2x fp16 mode (measured 0.59 ns/elem; GPSIMD
tensor ops share SBUF ports with DVE and slow it ~4x when co-active, so
they are not used). Filters are staged planar (9 per-tap planes): each
tap's product is a contiguous multiply where the 3x3 window shift is an
offset into the row-flat x tile; one fused multiply covers all 9 planes
via a [[W,3],[1,3],[1,FS]] read of x. The 9 planes are summed by a
4-instruction in-place binary tree. Filter border columns (taps that
would read x column padding) are zeroed host-side, so column wrap reads
multiply garbage by 0 and no x column padding is needed.
"""

import numpy as np

B, C, H, W = 8, 32, 256, 256
K = 3
N_CORES = 8
BAND = H // N_CORES            # 32 rows per core
RD = 8                         # rows per full super-tile
FS = RD * W                    # pixels per partition per super-tile (2048)
N_IMG = B * C                  # 256 images
P = 128
GROUPS = N_IMG // P            # 2
XLEN = (BAND + 2) * W + 2      # per-image padded x row storage (8706)

_CACHE = {}


def _strided_ap(tile_ap, dims, offset):
    """Copy of tile_ap with free dims replaced by [[step, count], ...]
    (element units) at element offset; partition dim preserved."""
    import bass_rust
    c = tile_ap.copy()
    part = list(c.ap)[0]
    c.ap = bass_rust.VecI64Pair([list(part)] + [list(d) for d in dims])
    c.offset = offset
    return c


def _build_module():
    import concourse.bacc as bacc
    import concourse.mybir as mybir
    from concourse.tile import TileContext

    fp16 = mybir.dt.float16
    add = mybir.AluOpType.add
    mult = mybir.AluOpType.mult

    nc = bacc.Bacc("TRN2", target_bir_lowering=False, debug=False)
    x_d = nc.dram_tensor("x_s", [N_IMG, XLEN], fp16,
                         kind="ExternalInput").ap()
    # planar taps: [img, tap, band_row, w]
    f_d = nc.dram_tensor("f_s", [N_IMG, K * K, BAND, W], fp16,
                         kind="ExternalInput").ap()
    o_d = nc.dram_tensor("o_s", [N_IMG, BAND, W], fp16,
                         kind="ExternalOutput").ap()

    # first super of the first group is split in half to shorten the
    # initial f-DMA ramp before the DVE can start
    supers = {0: [(0, RD // 2), (RD // 2, RD // 2), (RD, RD), (2 * RD, RD),
                  (3 * RD, RD)],
              1: [(s * RD, RD) for s in range(BAND // RD)]}

    with TileContext(nc) as tc:
        with (
            tc.tile_pool(name="xp", bufs=2) as xpool,
            tc.tile_pool(name="fp", bufs=2) as fpool,
            tc.tile_pool(name="pp", bufs=1) as ppool,
            tc.tile_pool(name="op", bufs=2) as opool,
        ):
            for g in range(GROUPS):
                p0 = g * P
                xt = xpool.tile([P, XLEN], fp16, tag="x")
                nc.scalar.dma_start(out=xt[:, :], in_=x_d[p0:p0 + P, :])
                for (r0, rd) in supers[g]:
                    fs = rd * W
                    ft = fpool.tile([P, K * K * FS], fp16, tag="f")
                    nc.sync.dma_start(
                        out=ft[:, 0:K * K * fs],
                        in_=f_d[p0:p0 + P, :, r0:r0 + rd, :],
                    )
                    pt = ppool.tile([P, K * K * FS], fp16, tag="p")
                    # fused 9-tap multiply: planes t=3i+j, contiguous out
                    xin = _strided_ap(xt[:, :], [[W, K], [1, K], [1, fs]],
                                      r0 * W)
                    fin = _strided_ap(ft[:, :], [[K * fs, K], [fs, K],
                                                 [1, fs]], 0)
                    pout = _strided_ap(pt[:, :], [[K * fs, K], [fs, K],
                                                  [1, fs]], 0)
                    nc.vector.tensor_tensor(pout, xin, fin, mult)
                    # binary-tree sum of the 9 planes, in place
                    nc.vector.tensor_tensor(
                        pt[:, 0:4 * fs], pt[:, 0:4 * fs],
                        pt[:, 4 * fs:8 * fs], add)
                    nc.vector.tensor_tensor(
                        pt[:, 0:2 * fs], pt[:, 0:2 * fs],
                        pt[:, 2 * fs:4 * fs], add)
                    nc.vector.tensor_tensor(
                        pt[:, 0:fs], pt[:, 0:fs], pt[:, fs:2 * fs], add)
                    ot = opool.tile([P, FS], fp16, tag="o")
                    nc.vector.tensor_tensor(
                        ot[:, 0:fs], pt[:, 0:fs], pt[:, 8 * fs:9 * fs], add)
                    nc.scalar.dma_start(
                        out=o_d[p0:p0 + P, r0:r0 + rd, :],
                        in_=ot[:, 0:fs],
                    )
    nc.compile()
    return nc


def _get_module():
    if "nc" not in _CACHE:
        _CACHE["nc"] = _build_module()
    return _CACHE["nc"]


def _shard_inputs(x, dynamic_filters):
    """Per-core input maps. x: [B,C,H,W] f32, filters: [B,C*9,H,W] f32."""
    xp = np.pad(x, ((0, 0), (0, 0), (1, 1), (0, 0))).astype(np.float16)
    # planar taps [img, t, H, W]; zero border cols (j=0 @ w=0, j=2 @ w=W-1)
    fp = np.ascontiguousarray(
        dynamic_filters.reshape(N_IMG, K * K, H, W)).astype(np.float16)
    fp[:, 0::3, :, 0] = 0.0
    fp[:, 2::3, :, W - 1] = 0.0

    in_maps = []
    for n in range(N_CORES):
        r = n * BAND
        xs = xp[:, :, r:r + BAND + 2, :].reshape(N_IMG, (BAND + 2) * W)
        xs_flat = np.zeros((N_IMG, XLEN), np.float16)
        xs_flat[:, 1:-1] = xs
        fs = np.ascontiguousarray(fp[:, :, r:r + BAND])
        in_maps.append({"x_s": xs_flat, "f_s": fs})
    return in_maps


def kernel(x, dynamic_filters, _trace=False):
    from concourse import bass_utils

    x = np.asarray(x, dtype=np.float32)
    dynamic_filters = np.asarray(dynamic_filters, dtype=np.float32)
    nc = _get_module()
    in_maps = _shard_inputs(x, dynamic_filters)
    res = bass_utils.run_bass_kernel_spmd(
        nc, in_maps, list(range(N_CORES)), trace=_trace)
    out = np.concatenate(
        [res.results[n]["o_s"].reshape(B, C, BAND, W).astype(np.float32)
         for n in range(N_CORES)],
        axis=2)
    _CACHE["last_exec_time_ns"] = res.exec_time_ns
    return out


# revision 4
# speedup vs baseline: 1.5259x; 1.0122x over previous
"""Dynamic 3x3 per-pixel filter (DynamicFilterLayer2D) on 8 Trainium2 cores.

Reference: out[b,c,h,w] = sum_{i,j in 3x3} xpad[b,c,h+i,w+j] * f[b,c,(3i+j),h,w]

Sharding: H is split into 8 bands of 32 rows; each core processes all
(b, c) images for its band (data parallel, 1-row halo). Per-core layout:
partitions = 128 (b,c) images (2 groups of 128), free dim = flat pixels.

All HBM traffic is fp16 (the 2e-2 rel-err gate leaves ~20x margin), which
halves DMA bytes versus fp32; per-core DMA roofline is ~371 GB/s.

Compute is pure DVE in 2x fp16 mode (measured 0.59 ns/elem; GPSIMD
tensor ops share SBUF ports with DVE and slow it ~4x when co-active, so
they are not used). Filters are staged planar (9 per-tap planes): each
tap's product is a contiguous multiply where the 3x3 window shift is an
offset into the row-flat x tile; one fused multiply covers all 9 planes
via a [[W,3],[1,3],[1,FS]] read of x. The 9 planes are summed by a
4-instruction in-place binary tree. Filter border columns (taps that
would read x column padding) are zeroed host-side, so column wrap reads
multiply garbage by 0 and no x column padding is needed.
"""

import numpy as np

B, C, H, W = 8, 32, 256, 256
K = 3
N_CORES = 8
BAND = H // N_CORES            # 32 rows per core
RD = 8                         # rows per full super-tile
FS = RD * W                    # pixels per partition per super-tile (2048)
N_IMG = B * C                  # 256 images
P = 128
GROUPS = N_IMG // P            # 2
XLEN = (BAND + 2) * W + 2      # per-image padded x row storage (8706)

_CACHE = {}


def _strided_ap(tile_ap, dims, offset):
    """Copy of tile_ap with free dims replaced by [[step, count], ...]
    (element units) at element offset; partition dim preserved."""
    import bass_rust
    c = tile_ap.copy()
    part = list(c.ap)[0]
    c.ap = bass_rust.VecI64Pair([list(part)] + [list(d) for d in dims])
    c.offset = offset
    return c


def _build_module():
    import concourse.bacc as bacc
    import concourse.mybir as mybir
    from concourse.tile import TileContext

    fp16 = mybir.dt.float16
    add = mybir.AluOpType.add
    mult = mybir.AluOpType.mult

    nc = bacc.Bacc("TRN2", target_bir_lowering=False, debug=False)
    x_d = nc.dram_tensor("x_s", [N_IMG, XLEN], fp16,
                         kind="ExternalInput").ap()
    # planar taps: [img, tap, band_row, w]
    f_d = nc.dram_tensor("f_s", [N_IMG, K * K, BAND, W], fp16,
                         kind="ExternalInput").ap()
    o_d = nc.dram_tensor("o_s", [N_IMG, BAND, W], fp16,
                         kind="ExternalOutput").ap()

    # tiny leading supers shorten the initial f/x-DMA ramp before the
    # DVE can start; small trailing supers shorten the drain tail
    supers = {0: [(0, 2), (2, 2), (4, 4), (RD, RD), (2 * RD, RD),
                  (3 * RD, RD)],
              1: [(0, RD), (RD, RD), (2 * RD, RD), (3 * RD, RD // 2),
                  (3 * RD + RD // 2, RD // 2)]}

    with TileContext(nc) as tc:
        with (
            tc.tile_pool(name="xp", bufs=3) as xpool,
            tc.tile_pool(name="fp", bufs=2) as fpool,
            tc.tile_pool(name="pp", bufs=1) as ppool,
            tc.tile_pool(name="op", bufs=2) as opool,
        ):
            for g in range(GROUPS):
                p0 = g * P
                for (r0, rd) in supers[g]:
                    fs = rd * W
                    xlen = (rd + 2) * W + 2
                    xt = xpool.tile([P, (RD + 2) * W + 2], fp16, tag="x")
                    nc.scalar.dma_start(
                        out=xt[:, 0:xlen],
                        in_=x_d[p0:p0 + P, r0 * W:r0 * W + xlen],
                    )
                    ft = fpool.tile([P, K * K * FS], fp16, tag="f")
                    nc.sync.dma_start(
                        out=ft[:, 0:K * K * fs],
                        in_=f_d[p0:p0 + P, :, r0:r0 + rd, :],
                    )
                    pt = ppool.tile([P, K * K * FS], fp16, tag="p")
                    # fused 9-tap multiply: planes t=3i+j, contiguous out
                    xin = _strided_ap(xt[:, :], [[W, K], [1, K], [1, fs]],
                                      0)
                    fin = _strided_ap(ft[:, :], [[K * fs, K], [fs, K],
                                                 [1, fs]], 0)
                    pout = _strided_ap(pt[:, :], [[K * fs, K], [fs, K],
                                                  [1, fs]], 0)
                    nc.vector.tensor_tensor(pout, xin, fin, mult)
                    # binary-tree sum of the 9 planes, in place
                    nc.vector.tensor_tensor(
                        pt[:, 0:4 * fs], pt[:, 0:4 * fs],
                        pt[:, 4 * fs:8 * fs], add)
                    nc.vector.tensor_tensor(
                        pt[:, 0:2 * fs], pt[:, 0:2 * fs],
                        pt[:, 2 * fs:4 * fs], add)
                    nc.vector.tensor_tensor(
                        pt[:, 0:fs], pt[:, 0:fs], pt[:, fs:2 * fs], add)
                    ot = opool.tile([P, FS], fp16, tag="o")
                    nc.vector.tensor_tensor(
                        ot[:, 0:fs], pt[:, 0:fs], pt[:, 8 * fs:9 * fs], add)
                    nc.scalar.dma_start(
                        out=o_d[p0:p0 + P, r0:r0 + rd, :],
                        in_=ot[:, 0:fs],
                    )
    nc.compile()
    return nc


def _get_module():
    if "nc" not in _CACHE:
        _CACHE["nc"] = _build_module()
    return _CACHE["nc"]


def _shard_inputs(x, dynamic_filters):
    """Per-core input maps. x: [B,C,H,W] f32, filters: [B,C*9,H,W] f32."""
    xp = np.pad(x, ((0, 0), (0, 0), (1, 1), (0, 0))).astype(np.float16)
    # planar taps [img, t, H, W]; zero border cols (j=0 @ w=0, j=2 @ w=W-1)
    fp = np.ascontiguousarray(
        dynamic_filters.reshape(N_IMG, K * K, H, W)).astype(np.float16)
    fp[:, 0::3, :, 0] = 0.0
    fp[:, 2::3, :, W - 1] = 0.0

    in_maps = []
    for n in range(N_CORES):
        r = n * BAND
        xs = xp[:, :, r:r + BAND + 2, :].reshape(N_IMG, (BAND + 2) * W)
        xs_flat = np.zeros((N_IMG, XLEN), np.float16)
        xs_flat[:, 1:-1] = xs
        fs = np.ascontiguousarray(fp[:, :, r:r + BAND])
        in_maps.append({"x_s": xs_flat, "f_s": fs})
    return in_maps


def kernel(x, dynamic_filters, _trace=False):
    from concourse import bass_utils

    x = np.asarray(x, dtype=np.float32)
    dynamic_filters = np.asarray(dynamic_filters, dtype=np.float32)
    nc = _get_module()
    in_maps = _shard_inputs(x, dynamic_filters)
    res = bass_utils.run_bass_kernel_spmd(
        nc, in_maps, list(range(N_CORES)), trace=_trace)
    out = np.concatenate(
        [res.results[n]["o_s"].reshape(B, C, BAND, W).astype(np.float32)
         for n in range(N_CORES)],
        axis=2)
    _CACHE["last_exec_time_ns"] = res.exec_time_ns
    return out
